# revision 9
# baseline (speedup 1.0000x reference)
"""3-layer GAT (heads=1) + global mean pool on 8 Trainium2 NeuronCores — v4.

Architecture (dst-sharded graph partition, PE one-hot aggregation):
- Edge-phase src rows come from native `dma_gather` (<=1024 idx/instruction;
  larger gathers crash the SWDGE ucode). Descriptor generation on the Pool
  Q7 costs ~7.5 ns/idx and is the kernel's hard floor, so everything else is
  arranged to hide beneath it:
  * self-loop edges never gather: each window's own H' rows stay resident in
    SBUF and contribute via an identity one-hot chunk;
  * the int16 index range is covered by TWO half tables assembled by two
    THIN (132-col) AllGathers (split so lo-half gathers start while the hi
    half is still collecting), each restrided 132->256 by a plain DRAM->DRAM
    HWDGE copy (hardware descriptors, zero Pool time);
  * gather groups are small (32 chunks) with 4 row buffers so the Pool never
    stalls on WAR.
- Attention math: ad[dst] via ad-column -> broadcast -> PE transpose ->
  one-hot x ad_row with a batched 3D tensor_reduce; leaky-relu as one fused
  scalar_tensor_tensor; exp is the only Act table function. The ee-scaled
  one-hot (matmul lhsT) is produced by the Act engine (copy with per-chunk
  scale column), as are phase-A PSUM->SBUF casts and the epilogue 1/deg
  scaling — the Vector engine only builds one-hots, the ad reduce and the
  small epilogue tail.
"""
import os
import sys
import time

import numpy as np

for p in ("/root/.axon_site", "/root/.axon_site/_ro/trn_rl_repo",
          "/root/.axon_site/_ro/pypackages", "/opt/trn_rl_repo", "/opt/pypackages"):
    if os.path.isdir(p) and p not in sys.path:
        sys.path.append(p)

from contextlib import ExitStack

import concourse.bass as bass
import concourse.mybir as mybir
import concourse.tile as tile
from concourse import bacc
from concourse.bass_utils import run_bass_kernel_spmd

N_NODES = 50000
N_GRAPHS = 512
NEG_SLOPE = 0.2
EPS = 1e-16
NC = 8
P = 128
D = 128
S_PAD = 6656            # padded per-core node slice (52 windows of 128)
NWIN = S_PAD // P
SH = S_PAD // 2         # 3328: per-core rows contributed to each half table
TW = 256                # fat table row stride (bf16)
HW = 132                # thin table row width [h|as|ad|one]
GW = 131                # matmul rhs width
VOCAB = 32000
GMAX = 32               # chunks per gather group (rows tile size)
SUB = 8                 # chunks per dma_gather instruction (1024 idxs max)

last_exec_time_ns = None
_COMPILED = {}


# ---------------------------------------------------------------- host prep
def _prep(node_ids, edge_index, batch):
    node_ids = node_ids.astype(np.int64)
    edge_index = edge_index.astype(np.int64)
    batch = batch.astype(np.int64)

    g_start = np.searchsorted(batch, np.arange(N_GRAPHS + 1))
    target = N_NODES / NC
    bounds = [0]
    for c in range(1, NC):
        want = c * target
        gi = np.searchsorted(g_start, want)
        cand = [g_start[max(gi - 1, 0)], g_start[min(gi, N_GRAPHS)]]
        bounds.append(int(min(cand, key=lambda v: abs(v - want))))
    bounds.append(N_NODES)
    bounds = np.array(bounds)
    assert np.all(np.diff(bounds) > 0) and np.all(np.diff(bounds) <= S_PAD)
    g_bounds = [int(batch[b]) if b < N_NODES else N_GRAPHS for b in bounds[:-1]]
    g_bounds.append(N_GRAPHS)

    # self-loops handled locally; only the 800K real edges go through gathers
    src_full = edge_index[0]
    dst_full = edge_index[1]
    src_core = np.searchsorted(bounds, src_full, side="right") - 1
    dst_core = np.searchsorted(bounds, dst_full, side="right") - 1
    src_j = src_full - bounds[src_core]          # local offset on owning core
    # half tables: half 0 holds rows j<SH of every core (row = c*SH + j),
    # half 1 holds rows j>=SH (row = c*SH + j-SH)
    src_half = (src_j >= SH).astype(np.int64)
    src_rel = (src_core * SH + np.where(src_half == 1, src_j - SH, src_j))
    dst_loc = dst_full - bounds[dst_core]

    cores = []
    cnt_half = np.zeros((NC, NWIN, 2), np.int64)
    for c in range(NC):
        sel = np.where(dst_core == c)[0]
        dloc = dst_loc[sel]
        srel = src_rel[sel]
        half = src_half[sel]
        w_of = dloc // P
        order = np.lexsort((half, w_of))
        dloc, srel, half, w_of = dloc[order], srel[order], half[order], w_of[order]
        for w in range(NWIN):
            m = w_of == w
            cnt_half[c, w, 0] = np.sum(m & (half == 0))
            cnt_half[c, w, 1] = np.sum(m & (half == 1))
        cores.append(dict(dloc=dloc, srel=srel, w_of=w_of, half=half,
                          s0=int(bounds[c]), S_c=int(bounds[c + 1] - bounds[c]),
                          g0=g_bounds[c], G_c=g_bounds[c + 1] - g_bounds[c]))
    assert max(c["G_c"] for c in cores) <= P

    k_wh = np.ceil(cnt_half.max(axis=0) / P).astype(np.int64)
    k_lo = [int(x) for x in k_wh[:, 0]]
    k_hi = [int(x) for x in k_wh[:, 1]]
    nch_lo, nch_hi = sum(k_lo), sum(k_hi)

    for c in cores:
        slots = {}
        for h, nch_s, k_s in ((0, nch_lo, k_lo), (1, nch_hi, k_hi)):
            src_sl = np.zeros((nch_s, P), np.int16)       # pad -> row 0
            dloc_pad = np.full((nch_s, P), -1.0, np.float32)
            pos = 0
            for w in range(NWIN):
                m = (c["w_of"] == w) & (c["half"] == h)
                sg = c["srel"][m]
                dl = c["dloc"][m] - w * P
                ne = sg.shape[0]
                flat = np.arange(ne)
                src_sl[pos + flat // P, flat % P] = sg.astype(np.int16)
                dloc_pad[pos + flat // P, flat % P] = dl.astype(np.float32)
                pos += k_s[w]
            assert pos == nch_s
            L = np.ascontiguousarray(src_sl.reshape(-1))
            wrapped = L.reshape(-1, 16).T.astype(np.int16)
            slots[h] = (np.ascontiguousarray(np.tile(wrapped, (8, 1))),
                        np.ascontiguousarray(dloc_pad.T))
        c["idx16_lo"], c["dloc_lo"] = slots[0]
        c["idx16_hi"], c["dloc_hi"] = slots[1]

        nid = np.full(S_PAD, node_ids[c["s0"]], np.int64)
        nid[:c["S_c"]] = node_ids[c["s0"]:c["s0"] + c["S_c"]]
        wrapped = nid.reshape(-1, 16).T.astype(np.int16)
        c["nid16"] = np.ascontiguousarray(np.tile(wrapped, (8, 1)))

        Pm = np.zeros((S_PAD, P), np.float32)
        bb = batch[c["s0"]:c["s0"] + c["S_c"]] - c["g0"]
        cnts = np.bincount(bb, minlength=c["G_c"]).astype(np.float32)
        w8 = 1.0 / np.maximum(cnts, 1.0)
        Pm[np.arange(c["S_c"]), bb] = w8[bb]
        c["p_pool"] = Pm
    return cores, k_lo, k_hi


def _groups(k_lo, k_hi):
    groups = []
    w0, clo0, chi0, alo, ahi = 0, 0, 0, 0, 0
    plo, phi = 0, 0
    for w in range(NWIN):
        cw = k_lo[w] + k_hi[w]
        if (alo + ahi) and (alo + ahi + cw > GMAX):
            groups.append((w0, w, clo0, alo, chi0, ahi))
            w0, clo0, chi0, alo, ahi = w, plo, phi, 0, 0
        alo += k_lo[w]
        ahi += k_hi[w]
        plo += k_lo[w]
        phi += k_hi[w]
    if alo + ahi:
        groups.append((w0, NWIN, clo0, alo, chi0, ahi))
    return groups


# ---------------------------------------------------------------- device
def _build(k_lo, k_hi):
    key = (tuple(k_lo), tuple(k_hi))
    if key in _COMPILED:
        return _COMPILED[key]
    f32, bf16 = mybir.dt.float32, mybir.dt.bfloat16
    i16 = mybir.dt.int16
    nc = bacc.Bacc("TRN2", num_devices=NC)
    nch_lo, nch_hi = sum(k_lo), sum(k_hi)
    kmax = max(max(k_lo), max(k_hi))
    groups = _groups(k_lo, k_hi)
    Act = mybir.ActivationFunctionType

    emb_d = nc.declare_dram_parameter("emb_bf", [VOCAB, D], bf16, isOutput=False)
    w_d = nc.declare_dram_parameter("w_all", [3, D, HW], bf16, isOutput=False)
    b_d = nc.declare_dram_parameter("b_bcast", [3, P, D], f32, isOutput=False)
    iota_d = nc.declare_dram_parameter("iota_rep", [P, kmax * P], bf16,
                                       isOutput=False)
    ident_d = nc.declare_dram_parameter("ident", [P, P], bf16, isOutput=False)
    nid_d = nc.declare_dram_parameter("nid16", [P, S_PAD // 16], i16,
                                      isOutput=False)
    ilo_d = nc.declare_dram_parameter("idx16_lo", [P, nch_lo * 8], i16,
                                      isOutput=False)
    ihi_d = nc.declare_dram_parameter("idx16_hi", [P, nch_hi * 8], i16,
                                      isOutput=False)
    dlo_d = nc.declare_dram_parameter("dloc_lo", [P, nch_lo], bf16,
                                      isOutput=False)
    dhi_d = nc.declare_dram_parameter("dloc_hi", [P, nch_hi], bf16,
                                      isOutput=False)
    pp_d = nc.declare_dram_parameter("p_pool", [S_PAD, P], bf16, isOutput=False)
    out_d = nc.declare_dram_parameter("out_pool", [P, D], f32, isOutput=True)

    # thin per-half shards -> thin Shared AllGather outputs -> fat local copies
    sh_thin = [nc.dram_tensor(f"sh_thin{h}", [SH, HW], bf16) for h in range(2)]
    full_thin = [nc.dram_tensor(f"full_thin{h}", [NC * SH, HW], bf16,
                                addr_space="Shared") for h in range(2)]
    full_fat = [nc.dram_tensor(f"full_fat{h}", [NC * SH, TW], bf16)
                for h in range(2)]

    with tile.TileContext(nc) as tc, ExitStack() as ctx:
        con = ctx.enter_context(tc.tile_pool(name="con", bufs=1))
        big = ctx.enter_context(tc.tile_pool(name="big", bufs=1))
        rows_p = ctx.enter_context(tc.tile_pool(name="rows", bufs=4))
        mee_p = ctx.enter_context(tc.tile_pool(name="mee", bufs=8))
        sml_p = ctx.enter_context(tc.tile_pool(name="sml", bufs=16))
        ups_p = ctx.enter_context(tc.tile_pool(name="ups", bufs=8))
        ph_p = ctx.enter_context(tc.tile_pool(name="ph", bufs=6))
        ps = ctx.enter_context(tc.tile_pool(name="ps", bufs=3, space="PSUM"))
        ps2 = ctx.enter_context(tc.tile_pool(name="ps2", bufs=2, space="PSUM"))
        ps1 = ctx.enter_context(tc.tile_pool(name="ps1", bufs=1, space="PSUM"))
        psp = ctx.enter_context(tc.tile_pool(name="psp", bufs=1, space="PSUM"))

        iota_t = con.tile([P, kmax * P], bf16)
        nc.sync.dma_start(iota_t[:], iota_d[:])
        ident_t = con.tile([P, P], bf16)
        nc.sync.dma_start(ident_t[:], ident_d[:])
        w_t, b_t = [], []
        for li in range(3):
            wt = con.tile([P, HW], bf16, tag=f"w{li}")
            nc.sync.dma_start(wt[:], w_d[li])
            w_t.append(wt)
            bt = con.tile([P, D], f32, tag=f"b{li}")
            nc.sync.dma_start(bt[:], b_d[li])
            b_t.append(bt)
        nid_t = con.tile([P, S_PAD // 16], i16)
        nc.sync.dma_start(nid_t[:], nid_d[:])
        ilo_t = con.tile([P, nch_lo * 8], i16)
        nc.sync.dma_start(ilo_t[:], ilo_d[:])
        ihi_t = con.tile([P, nch_hi * 8], i16)
        nc.sync.dma_start(ihi_t[:], ihi_d[:])
        dlo_t = con.tile([P, nch_lo], bf16)
        nc.sync.dma_start(dlo_t[:], dlo_d[:])
        dhi_t = con.tile([P, nch_hi], bf16)
        nc.sync.dma_start(dhi_t[:], dhi_d[:])

        x_sb = big.tile([P, S_PAD], bf16)       # node features, window-major
        h_loc = big.tile([P, NWIN * HW], bf16)  # own H' rows (self-loop chunks)

        # layer-1 input: embedding dma_gathers (<=1024 idxs each)
        x3g = x_sb[:].rearrange("p (w f) -> p w f", f=D)
        for g0 in range(0, NWIN, SUB):
            g1 = min(g0 + SUB, NWIN)
            nc.gpsimd.dma_gather(
                out_ap=x3g[:, g0:g1, :], in_ap=emb_d[:],
                idxs_ap=nid_t[:, g0 * 8:g1 * 8],
                num_idxs=(g1 - g0) * P, num_idxs_reg=(g1 - g0) * P,
                elem_size=D)

        for li in range(3):
            # ---- phase A: H'_local = X_local @ W', windows grouped by half
            for h in range(2):
                st3 = sh_thin[h].rearrange("(w p) f -> w p f", p=P)
                for wl in range(NWIN // 2):
                    w = h * (NWIN // 2) + wl
                    ws = slice(w * D, (w + 1) * D)
                    hcol = slice(w * HW, (w + 1) * HW)
                    xt_ps = ps2.tile([P, P], bf16, tag="tp")
                    nc.tensor.transpose(out=xt_ps[:], in_=x_sb[:, ws],
                                        identity=ident_t[:])
                    xt = ph_p.tile([P, P], bf16, tag="xt_sb")
                    nc.vector.tensor_copy(xt[:], xt_ps[:])
                    h_ps = ps1.tile([P, HW], f32, tag="hps")
                    nc.tensor.matmul(h_ps[:], lhsT=xt[:], rhs=w_t[li][:],
                                     start=True, stop=True)
                    nc.scalar.activation(h_loc[:, hcol], h_ps[:], Act.Copy)
                    nc.vector.memset(h_loc[:, w * HW + 130:w * HW + 131], 1.0)
                    nc.sync.dma_start(st3[wl, :, :], h_loc[:, hcol])
                # thin AllGather of this half, then HWDGE restride 132->256
                nc.gpsimd.collective_compute(
                    "AllGather", mybir.AluOpType.bypass,
                    replica_groups=[list(range(NC))],
                    ins=[sh_thin[h][:]], outs=[full_thin[h][:]])
                nc.sync.dma_start(full_fat[h][:, 0:HW], full_thin[h][:])

            # ---- edge phase
            for (w0, w1, clo0, Clo, chi0, Chi) in groups:
                rows_g = rows_p.tile([P, GMAX * TW], bf16, tag="rows")
                rows3 = rows_g[:].rearrange("p (c f) -> p c f", f=TW)
                for g0 in range(0, Clo, SUB):
                    g1 = min(g0 + SUB, Clo)
                    nc.gpsimd.dma_gather(
                        out_ap=rows3[:, g0:g1, :], in_ap=full_fat[0][:],
                        idxs_ap=ilo_t[:, (clo0 + g0) * 8:(clo0 + g1) * 8],
                        num_idxs=(g1 - g0) * P, num_idxs_reg=(g1 - g0) * P,
                        elem_size=TW)
                for g0 in range(0, Chi, SUB):
                    g1 = min(g0 + SUB, Chi)
                    nc.gpsimd.dma_gather(
                        out_ap=rows3[:, Clo + g0:Clo + g1, :],
                        in_ap=full_fat[1][:],
                        idxs_ap=ihi_t[:, (chi0 + g0) * 8:(chi0 + g1) * 8],
                        num_idxs=(g1 - g0) * P, num_idxs_reg=(g1 - g0) * P,
                        elem_size=TW)

                plo, phi = clo0, chi0
                for w in range(w0, w1):
                    ws = slice(w * D, (w + 1) * D)
                    hcol = w * HW
                    Ka, Kb = k_lo[w], k_hi[w]
                    K = Ka + Kb + 1            # + self-loop chunk
                    # ad_row via broadcast + PE transpose
                    adb = ph_p.tile([P, P], bf16, tag="adb")
                    nc.vector.tensor_copy(
                        adb[:],
                        h_loc[:, hcol + 129:hcol + 130].to_broadcast([P, P]))
                    adr_ps = ps2.tile([P, P], bf16, tag="tp")
                    nc.tensor.transpose(out=adr_ps[:], in_=adb[:],
                                        identity=ident_t[:])
                    ad_row = ph_p.tile([P, P], bf16, tag="adrow")
                    nc.vector.tensor_copy(ad_row[:], adr_ps[:])

                    psum = ps.tile([P, GW], f32, tag="edge")
                    # self-loop chunk: rows = own h window, one-hot = identity
                    s0t = sml_p.tile([P, 1], f32, tag="s0")
                    nc.vector.tensor_tensor(
                        out=s0t[:], in0=h_loc[:, hcol + 128:hcol + 129],
                        in1=h_loc[:, hcol + 129:hcol + 130],
                        op=mybir.AluOpType.add)
                    q0 = sml_p.tile([P, 1], f32, tag="q0")
                    nc.vector.scalar_tensor_tensor(
                        out=q0[:], in0=s0t[:], scalar=NEG_SLOPE, in1=s0t[:],
                        op0=mybir.AluOpType.mult, op1=mybir.AluOpType.max)
                    ee0 = sml_p.tile([P, 1], f32, tag="ee0")
                    nc.scalar.activation(ee0[:], q0[:], Act.Exp)
                    mee0 = mee_p.tile([P, P], bf16, tag="mee0")
                    nc.scalar.activation(mee0[:], ident_t[:], Act.Copy,
                                         scale=ee0[:, 0:1])
                    nc.tensor.matmul(psum[:], lhsT=mee0[:],
                                     rhs=h_loc[:, hcol:hcol + GW],
                                     start=True, stop=(K == 1))
                    kdone = 1
                    for (Ks, dloc_t_, pos0, tb0) in (
                            (Ka, dlo_t, plo, plo - clo0),
                            (Kb, dhi_t, phi, Clo + phi - chi0)):
                        if Ks == 0:
                            continue
                        oh = mee_p.tile([P, Ks * P], bf16, tag="oh")
                        oh3 = oh[:].rearrange("p (k j) -> p k j", j=P)
                        nc.vector.tensor_tensor(
                            out=oh3,
                            in0=iota_t[:, :Ks * P].rearrange(
                                "p (k j) -> p k j", j=P),
                            in1=dloc_t_[:, pos0:pos0 + Ks].unsqueeze(2)
                                .to_broadcast([P, Ks, P]),
                            op=mybir.AluOpType.is_equal)
                        tmp = mee_p.tile([P, Ks * P], bf16, tag="tmp")
                        tmp3 = tmp[:].rearrange("p (k j) -> p k j", j=P)
                        nc.vector.tensor_tensor(
                            out=tmp3, in0=oh3,
                            in1=ad_row[:].unsqueeze(1).to_broadcast([P, Ks, P]),
                            op=mybir.AluOpType.mult)
                        s = sml_p.tile([P, Ks], f32, tag="s")
                        nc.vector.tensor_reduce(
                            out=s[:].unsqueeze(2), in_=tmp3,
                            axis=mybir.AxisListType.X, op=mybir.AluOpType.add)
                        s2 = sml_p.tile([P, Ks], f32, tag="s2")
                        nc.vector.tensor_tensor(
                            out=s2[:].unsqueeze(2), in0=s[:].unsqueeze(2),
                            in1=rows3[:, tb0:tb0 + Ks, 128:129],
                            op=mybir.AluOpType.add)
                        q = sml_p.tile([P, Ks], f32, tag="q")
                        nc.vector.scalar_tensor_tensor(
                            out=q[:], in0=s2[:], scalar=NEG_SLOPE, in1=s2[:],
                            op0=mybir.AluOpType.mult, op1=mybir.AluOpType.max)
                        ee = sml_p.tile([P, Ks], f32, tag="ee")
                        nc.scalar.activation(ee[:], q[:], Act.Exp)
                        for k in range(Ks):
                            mee = mee_p.tile([P, P], bf16, tag="mee")
                            nc.scalar.activation(
                                mee[:], oh[:, k * P:(k + 1) * P], Act.Copy,
                                scale=ee[:, k:k + 1])
                            nc.tensor.matmul(
                                psum[:], lhsT=mee[:],
                                rhs=rows3[:, tb0 + k, 0:GW],
                                start=False, stop=(kdone + k == K - 1))
                        kdone += Ks
                    plo += Ka
                    phi += Kb

                    # epilogue
                    se = sml_p.tile([P, 1], f32, tag="se")
                    nc.vector.tensor_scalar(
                        out=se[:], in0=psum[:, 130:131], scalar1=EPS,
                        scalar2=None, op0=mybir.AluOpType.add)
                    rcp = sml_p.tile([P, 1], f32, tag="rcp")
                    nc.vector.reciprocal(rcp[:], se[:])
                    t1 = ups_p.tile([P, D], f32, tag="t1")
                    nc.scalar.activation(t1[:], psum[:, 0:D], Act.Copy,
                                         scale=rcp[:, 0:1])
                    t2 = ups_p.tile([P, D], f32, tag="t2")
                    nc.vector.tensor_tensor(out=t2[:], in0=t1[:],
                                            in1=b_t[li][:],
                                            op=mybir.AluOpType.add)
                    if li < 2:
                        nc.vector.tensor_scalar(
                            out=x_sb[:, ws], in0=t2[:], scalar1=0.0,
                            scalar2=None, op0=mybir.AluOpType.max)
                    else:
                        nc.vector.tensor_copy(x_sb[:, ws], t2[:])

        # ---- global mean pool
        pool_ps = psp.tile([P, D], f32, tag="pool")
        for w in range(NWIN):
            pt = ph_p.tile([P, P], bf16, tag="ppool")
            nc.sync.dma_start(pt[:], pp_d[w * P:(w + 1) * P, :])
            nc.tensor.matmul(pool_ps[:], lhsT=pt[:],
                             rhs=x_sb[:, w * D:(w + 1) * D],
                             start=(w == 0), stop=(w == NWIN - 1))
        po = ups_p.tile([P, D], f32, tag="po")
        nc.vector.tensor_copy(po[:], pool_ps[:])
        nc.sync.dma_start(out_d[:], po[:])

    n_inst = sum(len(bb.instructions) for bb in nc.main_func.blocks)
    print(f"[kernel] instructions: {n_inst}, chunks: {nch_lo}+{nch_hi}, "
          f"groups: {len(groups)}")
    if os.environ.get("KERNEL_TRACE_ONLY") == "1":
        return nc
    t0 = time.time()
    nc.compile()
    print(f"[kernel] bacc compile {time.time() - t0:.1f}s")
    _COMPILED[key] = nc
    return nc


def _install_ntff_hook():
    try:
        import contextlib
        import ctypes
        import types
        if "antenv.axon_hooks" in sys.modules:
            return True
        so = "/opt/axon/libaxon_pjrt.so"
        if not os.path.exists(so):
            return False
        lib = ctypes.CDLL(so)
        if not hasattr(lib, "axon_start_nrt_profile"):
            return False
        lib.axon_start_nrt_profile.argtypes = [ctypes.POINTER(ctypes.c_int64),
                                               ctypes.c_size_t]
        lib.axon_start_nrt_profile.restype = ctypes.c_int64
        lib.axon_stop_nrt_profile.argtypes = [ctypes.c_char_p]
        lib.axon_stop_nrt_profile.restype = ctypes.c_int64

        @contextlib.contextmanager
        def hook(output_dir, device_ids):
            import jax
            jax.devices()
            if device_ids:
                ids = (ctypes.c_int64 * len(device_ids))(*device_ids)
                rc = lib.axon_start_nrt_profile(ids, len(device_ids))
            else:
                rc = lib.axon_start_nrt_profile(None, 0)
            if rc != 0:
                raise RuntimeError(f"axon_start_nrt_profile rc={rc}")
            try:
                yield
            finally:
                lib.axon_stop_nrt_profile(str(output_dir).encode())

        m = types.ModuleType("antenv.axon_hooks")
        m.get_axon_ntff_profile_hook = lambda: hook
        m.set_axon_ntff_profile_hook = lambda h: None
        sys.modules["antenv.axon_hooks"] = m
        import antenv
        antenv.axon_hooks = m
        return True
    except Exception:
        return False


# ---------------------------------------------------------------- entry
def kernel(node_ids, edge_index, batch, emb,
           W1, as1, ad1, b1, W2, as2, ad2, b2, W3, as3, ad3, b3):
    global last_exec_time_ns
    cores, k_lo, k_hi = _prep(np.asarray(node_ids), np.asarray(edge_index),
                              np.asarray(batch))
    kmax = max(max(k_lo), max(k_hi))

    def to_bf16(a):
        import jax.numpy as jnp
        return np.asarray(jnp.asarray(np.asarray(a, np.float32), jnp.bfloat16))

    w_all = np.zeros((3, D, HW), np.float32)
    b_bc = np.zeros((3, P, D), np.float32)
    for i, (W, a_s, a_d, b) in enumerate([(W1, as1, ad1, b1), (W2, as2, ad2, b2),
                                          (W3, as3, ad3, b3)]):
        W = np.asarray(W, np.float32)
        w_all[i, :, :D] = W
        w_all[i, :, 128] = W @ np.asarray(a_s, np.float32)
        w_all[i, :, 129] = W @ np.asarray(a_d, np.float32)
        b_bc[i] = np.tile(np.asarray(b, np.float32)[None, :], (P, 1))
    iota_rep = np.tile(np.arange(P, dtype=np.float32)[None, :], (P, kmax))
    ident = np.eye(P, dtype=np.float32)

    emb_bf = to_bf16(emb)
    in_maps = []
    for c in cores:
        in_maps.append(dict(emb_bf=emb_bf, w_all=to_bf16(w_all), b_bcast=b_bc,
                            iota_rep=to_bf16(iota_rep), ident=to_bf16(ident),
                            nid16=c["nid16"], idx16_lo=c["idx16_lo"],
                            idx16_hi=c["idx16_hi"],
                            dloc_lo=to_bf16(c["dloc_lo"]),
                            dloc_hi=to_bf16(c["dloc_hi"]),
                            p_pool=to_bf16(c["p_pool"])))

    nc = _build(k_lo, k_hi)
    trace = _install_ntff_hook() and os.environ.get("KERNEL_NO_TRACE") != "1"
    res = run_bass_kernel_spmd(nc, in_maps, list(range(NC)), trace=trace)
    last_exec_time_ns = res.exec_time_ns

    out = np.zeros((N_GRAPHS, D), np.float32)
    for ci, c in enumerate(cores):
        out[c["g0"]:c["g0"] + c["G_c"]] = res.results[ci]["out_pool"][:c["G_c"]]
    return out
